# revision 9
# baseline (speedup 1.0000x reference)
"""nn_CART_69355131895963 Trainium2 Bass kernel.

reference:
    BatchNorm1d(train-mode batch stats) -> per-tree sparsemax feature
    selection (einsum bf,tfs->tbs) -> sigmoid(xp - cut) -> per-tree
    [S,S] MLP layer + relu -> per-tree [S,O] layer -> mean over trees of
    o2 * tw.

Strategy (8 NeuronCores, batch-sharded 8192 rows/core):
  Host (O(params) only): sparsemax(fsm) -> P2 [F,TS]; fold gamma into the
    BN scale, tw/T into W2, build block-diagonal W1 (4 trees/group) and
    stacked W2'; lay every small parameter out for direct SBUF use.
  Device phase 1: stream x fp32 -> cast fp16 (GPSIMD) -> stats matmuls on
    PE (batch-sum via ones-lhsT, sum-of-squares via per-tile self-matmul
    diagonal) -> write x16 to DRAM scratch -> DMA-transpose to xT16 [f,b].
  Device phase 1.5: AllReduce the [2,F] stats across the 8 cores, finish
    mean/var -> rsqrt -> fold into P2a (fp16) and the sigmoid bias.
  Device phase 2 (software-pipelined over 64 (chunk, ts-tile) steps):
    s1: xp_tile = P2a^T @ xT  (PE, fp16, fp32 accum)
    ACT: score = sigmoid(xp + biasA)        (PSUM -> SBUF fp16)
    s2: z2 = W1bd^T @ score                 (PE)
    DVE: o1 = max(z2 + b1, 0)               (PSUM -> SBUF fp16)
    s3: out[16,b] += W2'^T @ o1             (PE, accumulated over 8 groups)
    evict: out + bout (ACT/DVE split) -> DMA to DRAM [16, b]
  Host: concat per-core [16, 8192] outputs along b, transpose -> [B, 16].
"""

import numpy as np

import concourse.tile as tile
from concourse import bacc, mybir
from concourse.bass_utils import run_bass_kernel_spmd

f16 = mybir.dt.float16
f32 = mybir.dt.float32
AF = mybir.ActivationFunctionType
ALU = mybir.AluOpType

N_CORES = 8
B_TOTAL = 65536
BS = B_TOTAL // N_CORES     # 8192 rows per core
F = 256
T = 32
S = 32
O = 16
TS = T * S                  # 1024
NFT = F // 128              # 2 feature tiles
NM = TS // 128              # 8 ts-tiles (tree groups of 4)
BN_EPS = 1e-5
CHUNK = 1024
Q = CHUNK // 512
NCH = BS // CHUNK
ROWS1 = 2048                # phase-1 chunk rows
NCH1 = BS // ROWS1
SUB1 = ROWS1 // 128


def _sparsemax_cols(z):
    """sparsemax along axis 0 of z [F, C] (float64)."""
    zs = np.sort(z, axis=0)[::-1]
    k = np.arange(1, z.shape[0] + 1)[:, None]
    cs = np.cumsum(zs, axis=0)
    support = (1.0 + k * zs) > cs
    ksup = support.sum(0)
    tau = (cs[ksup - 1, np.arange(z.shape[1])] - 1.0) / ksup
    return np.maximum(z - tau, 0.0)


def _host_prep(gamma, beta, fsm, cut, W1, b1, W2, b2, tw):
    P2 = _sparsemax_cols(
        fsm.astype(np.float64).transpose(1, 0, 2).reshape(F, TS)
    ).astype(np.float32)
    p2raw = P2.reshape(NFT, 128, TS).transpose(1, 0, 2).copy()
    cutv = cut.reshape(TS).reshape(NM, 128).T.copy().astype(np.float32)
    b1v = b1.reshape(TS).reshape(NM, 128).T.copy().astype(np.float32)

    w1bd = np.zeros((NM, 128, 128), dtype=np.float32)
    for g in range(NM):
        for i in range(4):
            w1bd[g, 32 * i:32 * i + 32, 32 * i:32 * i + 32] = W1[4 * g + i]
    w1bd = w1bd.transpose(1, 0, 2).astype(np.float16).copy()

    w2f = (W2 * (tw / T)).reshape(TS, O).astype(np.float32) \
        .reshape(NM, 128, O).transpose(1, 0, 2).astype(np.float16).copy()
    bout = (b2 * (tw / T)).sum(0).reshape(O, 1).astype(np.float32)

    gamma2 = gamma.reshape(NFT, 128).T.copy().astype(np.float32)
    beta2 = beta.reshape(NFT, 128).T.copy().astype(np.float32)
    eye = np.eye(128, dtype=np.float32)
    ones16 = np.ones((128, 1), dtype=np.float16)
    return dict(p2raw=p2raw, cutv=cutv, b1v=b1v, w1bd=w1bd, w2f=w2f,
                bout=bout, gamma2=gamma2, beta2=beta2, eye=eye, ones16=ones16)


def build_program(repeat=1, single_core_sim=False):
    """Trace + compile the SPMD Bass program (identical on all 8 cores).

    single_core_sim=True builds the same per-core program with the
    cross-core AllReduce elided (for cost-model simulation only).
    """
    ncores = 1 if single_core_sim else N_CORES
    nc = bacc.Bacc("TRN2", target_bir_lowering=False, debug=False,
                   num_devices=ncores)
    X = nc.dram_tensor("x", [BS, F], f32, kind="ExternalInput").ap()
    P2RAW = nc.dram_tensor("p2raw", [128, NFT, TS], f32, kind="ExternalInput").ap()
    CUTV = nc.dram_tensor("cutv", [128, NM], f32, kind="ExternalInput").ap()
    B1V = nc.dram_tensor("b1v", [128, NM], f32, kind="ExternalInput").ap()
    W1BD = nc.dram_tensor("w1bd", [128, NM, 128], f16, kind="ExternalInput").ap()
    W2F = nc.dram_tensor("w2f", [128, NM, O], f16, kind="ExternalInput").ap()
    BOUT = nc.dram_tensor("bout", [O, 1], f32, kind="ExternalInput").ap()
    GAMMA2 = nc.dram_tensor("gamma2", [128, NFT], f32, kind="ExternalInput").ap()
    BETA2 = nc.dram_tensor("beta2", [128, NFT], f32, kind="ExternalInput").ap()
    EYE = nc.dram_tensor("eye", [128, 128], f32, kind="ExternalInput").ap()
    ONES16 = nc.dram_tensor("ones16", [128, 1], f16, kind="ExternalInput").ap()
    OUT = nc.dram_tensor("out", [O, BS], f32, kind="ExternalOutput").ap()

    Xv = X.rearrange("(n p) f -> p n f", p=128)

    with tile.TileContext(nc) as tc:
        with tc.tile_pool(name="const", bufs=1) as pc, \
             tc.tile_pool(name="xt", bufs=1) as pxt, \
             tc.tile_pool(name="dram", bufs=1, space="DRAM") as pdram:

            def load_const(name, shape, dt, src):
                t = pc.tile(shape, dt, name=name)
                nc.sync.dma_start(t[:], src[:])
                return t

            p2raw = load_const("p2raw_sb", [128, NFT, TS], f32, P2RAW)
            cutv = load_const("cutv_sb", [128, NM], f32, CUTV)
            b1v = load_const("b1v_sb", [128, NM], f32, B1V)
            w1bd = load_const("w1bd_sb", [128, NM, 128], f16, W1BD)
            w2f = load_const("w2f_sb", [128, NM, O], f16, W2F)
            bout = load_const("bout_sb", [O, 1], f32, BOUT)
            gamma2 = load_const("gamma2_sb", [128, NFT], f32, GAMMA2)
            beta2 = load_const("beta2_sb", [128, NFT], f32, BETA2)
            eye = load_const("eye_sb", [128, 128], f32, EYE)
            ones16 = load_const("ones16_sb", [128, 1], f16, ONES16)
            eye16 = pc.tile([128, 128], f16, name="eye16")
            nc.vector.tensor_copy(eye16[:], eye[:])

            xT = [pxt.tile([128, BS], f16, tag=f"xt{i}", name=f"xt{i}")
                  for i in range(NFT)]
            x16d = pdram.tile([NFT, BS, 128], f16)

            def body_once():
                # ---------- phase 1: load, cast fp16, stats, transpose ----
                with tc.tile_pool(name="ph1", bufs=2) as p1, \
                     tc.tile_pool(name="ph1psum", bufs=1, space="PSUM") as pst:
                    sumP = pst.tile([1, F], f32, name="sumP")
                    covP = [pst.tile([128, 128], f32, tag=f"cov{i}",
                                     name=f"cov{i}") for i in range(NFT)]
                    for c in range(NCH1):
                        x32 = p1.tile([128, SUB1, F], f32, tag="x32",
                                      name="x32")
                        nc.sync.dma_start(x32[:],
                                          Xv[:, c * SUB1:(c + 1) * SUB1, :])
                        x16 = p1.tile([128, SUB1, F], f16, tag="x16",
                                      name="x16")
                        nc.gpsimd.tensor_copy(x16[:], x32[:])
                        for i in range(NFT):
                            nc.sync.dma_start(
                                x16d[i, c * ROWS1:(c + 1) * ROWS1, :]
                                  .rearrange("(a p) f -> p a f", p=128),
                                x16[:, :, 128 * i:128 * (i + 1)])
                        for a in range(SUB1):
                            first = (c == 0 and a == 0)
                            last = (c == NCH1 - 1 and a == SUB1 - 1)
                            nc.tensor.matmul(sumP[:], ones16[:],
                                             x16[:, a, :], start=first,
                                             stop=last, skip_group_check=True)
                            for i in range(NFT):
                                sl = x16[:, a, 128 * i:128 * (i + 1)]
                                nc.tensor.matmul(covP[i][:], sl, sl,
                                                 start=first, stop=last,
                                                 skip_group_check=True)
                    stat_sb = pc.tile([128, NFT, 2], f32, name="stat_sb")
                    sum_sb = pc.tile([1, F], f32, name="sum_sb")
                    nc.vector.tensor_copy(sum_sb[:], sumP[:])
                    for i in range(NFT):
                        tmp = p1.tile([128, 128], f32, tag="dtmp", name="dtmp")
                        nc.vector.tensor_tensor(tmp[:], covP[i][:], eye[:],
                                                op=ALU.mult)
                        nc.vector.reduce_sum(stat_sb[:, i, 1:2], tmp[:],
                                             axis=mybir.AxisListType.X)

                # transposes run while the collective is in flight
                for c in range(NCH1):
                    for i in range(NFT):
                        nc.sync.dma_start_transpose(
                            out=xT[i][:, c * ROWS1:(c + 1) * ROWS1],
                            in_=x16d[i, c * ROWS1:(c + 1) * ROWS1, :])

                # ---------- phase 1.5: all-reduce + BN fold ----------
                ccin = pdram.tile([2, F], f32, name="ccin")
                ccout = pdram.tile([2, F], f32, name="ccout")
                nc.sync.dma_start(ccin[0:1, :], sum_sb[:])
                nc.sync.dma_start(
                    ccin[1:2, :].rearrange("1 (i p) -> p i 1", p=128),
                    stat_sb[:, :, 1:2])
                if single_core_sim:
                    nc.gpsimd.dma_start(ccout[:], ccin[:])
                else:
                    nc.gpsimd.collective_compute(
                        "AllReduce", ALU.add,
                        replica_groups=[list(range(N_CORES))],
                        ins=[ccin.opt()], outs=[ccout.opt()])
                nc.sync.dma_start(
                    stat_sb[:, :, 0:1].rearrange("p i 1 -> p i"),
                    ccout[0:1, :].rearrange("1 (i p) -> p i", p=128))
                nc.sync.dma_start(
                    stat_sb[:, :, 1:2].rearrange("p i 1 -> p i"),
                    ccout[1:2, :].rearrange("1 (i p) -> p i", p=128))

                mean = pc.tile([128, NFT], f32, name="mean")
                nc.vector.tensor_scalar(mean[:], stat_sb[:, :, 0],
                                        1.0 / B_TOTAL, None, op0=ALU.mult)
                ex2 = pc.tile([128, NFT], f32, name="ex2")
                nc.vector.tensor_scalar(ex2[:], stat_sb[:, :, 1],
                                        1.0 / B_TOTAL, None, op0=ALU.mult)
                var = pc.tile([128, NFT], f32, name="var")
                nc.vector.tensor_tensor(var[:], mean[:], mean[:], op=ALU.mult)
                nc.vector.tensor_tensor(var[:], ex2[:], var[:],
                                        op=ALU.subtract)
                eps = pc.tile([128, 1], f32, name="eps")
                nc.vector.memset(eps[:], BN_EPS)
                se = pc.tile([128, NFT], f32, name="se")
                nc.scalar.activation(se[:], var[:], AF.Sqrt, bias=eps[:])
                sinv = pc.tile([128, NFT], f32, name="sinv")
                nc.vector.reciprocal(sinv[:], se[:])
                av = pc.tile([128, NFT], f32, name="av")
                nc.vector.tensor_tensor(av[:], sinv[:], gamma2[:],
                                        op=ALU.mult)
                cv = pc.tile([128, NFT], f32, name="cv")
                nc.vector.tensor_tensor(cv[:], mean[:], av[:], op=ALU.mult)
                nc.vector.tensor_tensor(cv[:], beta2[:], cv[:],
                                        op=ALU.subtract)

                p2a = [pc.tile([128, TS], f16, tag=f"p2a{i}", name=f"p2a{i}")
                       for i in range(NFT)]
                for i in range(NFT):
                    nc.vector.tensor_scalar(p2a[i][:], p2raw[:, i, :],
                                            av[:, i:i + 1], None,
                                            op0=ALU.mult)
                biasA = pc.tile([128, NM], f32, name="biasA")
                with tc.tile_pool(name="dps", bufs=1, space="PSUM") as pdp:
                    dP = pdp.tile([128, NM], f32, name="dP")
                    for m in range(NM):
                        for i in range(NFT):
                            nc.tensor.matmul(
                                dP[:, m:m + 1],
                                p2raw[:, i, 128 * m:128 * (m + 1)],
                                cv[:, i:i + 1],
                                start=(i == 0), stop=(i == NFT - 1))
                    nc.vector.tensor_tensor(biasA[:], dP[:], cutv[:],
                                            op=ALU.subtract)

                # ---------- phase 2: software-pipelined tree forest ------
                with tc.tile_pool(name="z", bufs=3, space="PSUM") as pz, \
                     tc.tile_pool(name="outp", bufs=2, space="PSUM") as pop, \
                     tc.tile_pool(name="sc", bufs=3) as psc, \
                     tc.tile_pool(name="o1", bufs=3) as po1, \
                     tc.tile_pool(name="osb", bufs=3) as pos:
                    NJ = NCH * NM
                    scs, o1s, outPs = {}, {}, {}

                    def stageA(j):
                        c, m = divmod(j, NM)
                        zp = pz.tile([128, CHUNK], f32, tag="z", name="zp")
                        for i in range(NFT):
                            for q in range(Q):
                                nc.tensor.matmul(
                                    zp[:, 512 * q:512 * (q + 1)],
                                    p2a[i][:, 128 * m:128 * (m + 1)],
                                    xT[i][:, c * CHUNK + 512 * q:
                                          c * CHUNK + 512 * (q + 1)],
                                    start=(i == 0), stop=(i == NFT - 1),
                                    skip_group_check=True)
                        sc = psc.tile([128, CHUNK], f16, tag="sc", name="sc")
                        nc.scalar.activation(sc[:], zp[:], AF.Sigmoid,
                                             bias=biasA[:, m:m + 1])
                        scs[j] = sc

                    def stageB(j):
                        c, m = divmod(j, NM)
                        sc = scs.pop(j)
                        z2 = pz.tile([128, CHUNK], f32, tag="z", name="z2")
                        for q in range(Q):
                            nc.tensor.matmul(z2[:, 512 * q:512 * (q + 1)],
                                             w1bd[:, m, :],
                                             sc[:, 512 * q:512 * (q + 1)],
                                             start=True, stop=True)
                        o1 = po1.tile([128, CHUNK], f16, tag="o1", name="o1")
                        nc.vector.tensor_scalar(o1[:], z2[:], b1v[:, m:m + 1],
                                                0.0, op0=ALU.add, op1=ALU.max)
                        o1s[j] = o1

                    def stageC(j):
                        c, m = divmod(j, NM)
                        if m == 0:
                            outPs[c] = pop.tile([128, 512], f32, tag="outp",
                                                name=f"outp{c}")
                        o1 = o1s.pop(j)
                        for q in range(Q):
                            nc.tensor.matmul(
                                outPs[c][32 * q:32 * q + O, :], w2f[:, m, :],
                                o1[:, 512 * q:512 * (q + 1)],
                                start=(m == 0), stop=(m == NM - 1),
                                skip_group_check=True,
                                tile_position=(0, 32 * q))
                        if m == NM - 1:
                            for q in range(Q):
                                osb = pos.tile([O, 512], f32, tag="osb",
                                               name="osb")
                                src_ap = outPs[c][32 * q:32 * q + O, :]
                                if q % 2 == 0:
                                    nc.scalar.activation(osb[:], src_ap,
                                                         AF.Identity,
                                                         bias=bout[:])
                                else:
                                    nc.vector.tensor_scalar(
                                        osb[:], src_ap, bout[:],
                                        None, op0=ALU.add)
                                nc.sync.dma_start(
                                    OUT[:, c * CHUNK + 512 * q:
                                        c * CHUNK + 512 * (q + 1)], osb[:])
                            del outPs[c]

                    for j in range(NJ + 2):
                        if j < NJ:
                            stageA(j)
                        if 1 <= j < NJ + 1:
                            stageB(j - 1)
                        if j >= 2:
                            stageC(j - 2)

            for _rep in range(repeat):
                body_once()
    nc.compile()
    return nc


_NC_CACHE = {}


def _get_program(repeat=1):
    if repeat not in _NC_CACHE:
        _NC_CACHE[repeat] = build_program(repeat)
    return _NC_CACHE[repeat]


def make_in_maps(inputs):
    x = np.ascontiguousarray(inputs["x"], dtype=np.float32)
    params = _host_prep(np.asarray(inputs["gamma"]), np.asarray(inputs["beta"]),
                        np.asarray(inputs["fsm"]), np.asarray(inputs["cut"]),
                        np.asarray(inputs["W1"]), np.asarray(inputs["b1"]),
                        np.asarray(inputs["W2"]), np.asarray(inputs["b2"]),
                        np.asarray(inputs["tw"]))
    return [{"x": x[c * BS:(c + 1) * BS], **params} for c in range(N_CORES)]


def kernel(x, gamma, beta, fsm, cut, W1, b1, W2, b2, tw):
    """Full unsharded inputs in, full [B, O] float32 output out."""
    inputs = dict(x=x, gamma=gamma, beta=beta, fsm=fsm, cut=cut, W1=W1,
                  b1=b1, W2=W2, b2=b2, tw=tw)
    nc = _get_program(repeat=1)
    in_maps = make_in_maps(inputs)
    res = run_bass_kernel_spmd(nc, in_maps, core_ids=list(range(N_CORES)))
    out = np.concatenate([res.results[c]["out"] for c in range(N_CORES)],
                         axis=1)
    return np.ascontiguousarray(out.T, dtype=np.float32)
